# revision 26
# baseline (speedup 1.0000x reference)
"""CRF negative log-likelihood loss on 8 Trainium2 NeuronCores.

Strategy: data-parallel over batch (64 sequences per core) with an
M=9-segment split of each sequence cutting the serial scan depth to 114
device slots. Boundaries BND[j] = j*114 (last segment 912..1024).

  alpha: exact forward chain over seg1 (48 states + hold for short L)
  x_j = T_j . 1         (fwd-seeded through middle segment j)
  y_j = T_j^T . e_stop  (bwd-seeded; doubles as the exact tail for batches
                         whose L falls inside segment j via a src state
                         injected at tau=L)
  b = A^T G_{913} beta_{913} (exact backward chain over the last segment)

Products of >=114 positive transfer matrices are numerically rank-1
(Birkhoff contraction), so T_j w ~ x_j (y_j.w)/(y_j.1) and the partition
function composes from host-side dots telescoping across segments:
  Z ~ (b.x_{M-1}) prod_j [(y_j.x_{j-1})/(y_j.1)] (y_2.alpha)
truncated at the segment containing L.

Packing: the 2(M-1) half-chains form M-1=8 packs sharing one block [98,98]
bf16 stationary: P1=[alpha|y2], Pj=[x_j|y_{j+1}], P8=[x_8|b]. Packs are
grouped into TWO super-chains of 4 packs, each a [98, 256] datapath: one
PE matmul + one wide DVE Hadamard per slot (the wide TT amortizes the
fixed 125ns PSUM-access cost over 256 columns). The two super-chains are
independent and interleave on the engines, hiding cross-engine sync
latency. All chains run in the exponential domain (bf16, fp32 PSUM) with
host-precomputed per-(batch,step) shifts; the gold path score is a cheap
host gather.
"""
import numpy as np
import ml_dtypes
from contextlib import ExitStack

import concourse.bacc as bacc
import concourse.bass as bass
import concourse.tile as tile
from concourse import mybir
from concourse.bass_utils import run_bass_kernel_spmd

B, T, K = 512, 1024, 48
START, STOP = 46, 47
NEG = -10000.0
HOLD = 48
SRCL = 48
KF = 49
K2 = 98
NCORES = 8
BC = B // NCORES    # 64
M = 9               # segments
NP = M - 1          # packs (8)
NG = NP // 2        # packs per super-chain (4)
W = NG * BC         # super-chain width (256)
D = -(-1025 // M)   # 114 device slots
BND = [j * D for j in range(M)] + [1024]
CH = 8              # slots per emission chunk (114 = 3*38)

_nc_cache = {}


def _build_module(d_slots=D, ch=CH):
    key = ("nc", d_slots, ch)
    if key in _nc_cache:
        return _nc_cache[key]
    nc = bacc.Bacc(
        "TRN2",
        target_bir_lowering=False,
        debug=False,
        enable_asserts=False,
        num_devices=NCORES,
    )
    f32 = mybir.dt.float32
    bf16 = mybir.dt.bfloat16
    fp8 = mybir.dt.float8e4
    e_dram = nc.dram_tensor("etil", [K2, K2], bf16, kind="ExternalInput").ap()
    gA_dram = nc.dram_tensor("emisA", [K2, d_slots, W], fp8, kind="ExternalInput").ap()
    gB_dram = nc.dram_tensor("emisB", [K2, d_slots, W], fp8, kind="ExternalInput").ap()
    w0_dram = nc.dram_tensor("w0", [K2, 2 * W], bf16, kind="ExternalInput").ap()
    o_dram = nc.dram_tensor("uout", [K2, 2 * W], bf16, kind="ExternalOutput").ap()

    with tile.TileContext(nc) as tc:
        with ExitStack() as ctx:
            const = ctx.enter_context(tc.tile_pool(name="const", bufs=1))
            wpool = ctx.enter_context(tc.tile_pool(name="wp", bufs=4))
            gexp_p = ctx.enter_context(tc.tile_pool(name="gexp", bufs=3))
            psum_p = ctx.enter_context(tc.tile_pool(name="ps", bufs=2, space="PSUM"))

            etile = const.tile([K2, K2], bf16)
            nc.sync.dma_start(out=etile, in_=e_dram)

            wA = const.tile([K2, W], bf16)
            nc.sync.dma_start(out=wA, in_=w0_dram[:, 0:W])
            wB = const.tile([K2, W], bf16)
            nc.scalar.dma_start(out=wB, in_=w0_dram[:, W : 2 * W])

            outT = const.tile([K2, 2 * W], bf16)
            outA = outT[:, 0:W]
            outB = outT[:, W : 2 * W]

            nstep = 0
            sched = [3, 6, 8] + [ch] * 100
            while nstep < d_slots:
                ns = min(sched.pop(0), d_slots - nstep)
                geA = gexp_p.tile([K2, ch, W], fp8, tag="geA")
                nc.sync.dma_start(
                    out=geA[:, :ns, :], in_=gA_dram[:, nstep : nstep + ns, :]
                )
                geB = gexp_p.tile([K2, ch, W], fp8, tag="geB")
                nc.scalar.dma_start(
                    out=geB[:, :ns, :], in_=gB_dram[:, nstep : nstep + ns, :]
                )
                for s in range(ns):
                    last = nstep + s == d_slots - 1
                    psA = psum_p.tile([K2, W], f32, tag="psA")
                    for c0 in range(0, W, 512):
                        c1 = min(c0 + 512, W)
                        nc.tensor.matmul(
                            psA[:, c0:c1], etile, wA[:, c0:c1], start=True, stop=True
                        )
                    psB = psum_p.tile([K2, W], f32, tag="psB")
                    for c0 in range(0, W, 512):
                        c1 = min(c0 + 512, W)
                        nc.tensor.matmul(
                            psB[:, c0:c1], etile, wB[:, c0:c1], start=True, stop=True
                        )
                    if last:
                        nc.vector.tensor_mul(outA, psA, geA[:, s, :])
                        nc.vector.tensor_mul(outB, psB, geB[:, s, :])
                    else:
                        wAn = wpool.tile([K2, W], bf16, tag="wA")
                        nc.vector.tensor_mul(wAn, psA, geA[:, s, :])
                        wA = wAn
                        wBn = wpool.tile([K2, W], bf16, tag="wB")
                        nc.vector.tensor_mul(wBn, psB, geB[:, s, :])
                        wB = wBn
                nstep += ns
            nc.sync.dma_start(out=o_dram, in_=outT)

    nc.compile()
    _nc_cache[key] = nc
    return nc


def _host_prep(feats, seq_len, trans):
    feats = np.ascontiguousarray(feats, dtype=np.float32)
    seq_len = np.asarray(seq_len, dtype=np.int64)
    trans = np.asarray(trans, dtype=np.float32)
    L = seq_len

    mx = feats.max(axis=2)
    E64 = np.exp(trans.astype(np.float64)).T

    drift = []
    for b in range(6):
        fv = np.full(K, NEG, dtype=np.float64)
        fv[START] = 0.0
        Lb = int(L[b])
        for t in range(min(Lb, 256)):
            m = fv.max()
            wv = np.exp(fv - m)
            with np.errstate(divide="ignore"):
                fv = np.log(E64.T @ wv) + m + feats[b, t]
            drift.append((fv.max() - m) - mx[b, t])
    mu = float(np.mean(drift))

    driftb = []
    nb = 0
    for b in range(B):
        Lb = int(L[b])
        if Lb < 700:
            continue
        nb += 1
        if nb > 6:
            break
        bv = trans[STOP, :].astype(np.float64).copy()
        for t in range(Lb, Lb - 200, -1):
            m = bv.max()
            wv = np.exp(bv - m)
            gv = np.exp(feats[b, t - 1].astype(np.float64))
            with np.errstate(divide="ignore"):
                bv = np.log(E64 @ (gv * wv)) + m
            driftb.append((bv.max() - m) - mx[b, t - 1])
    mub = float(np.mean(driftb)) if driftb else mu

    c = mx + mu
    cb = mx + mub
    Ccum = np.cumsum(c, axis=1, dtype=np.float64)
    Ccumb = np.cumsum(cb, axis=1, dtype=np.float64)
    estop = np.exp(trans[STOP, :K].astype(np.float64))
    estop32 = estop.astype(np.float32)

    n_ = np.arange(1, D + 1)
    gfall = np.exp(feats - c[:, :, None])
    gball = np.exp(feats - cb[:, :, None])

    ga = np.zeros((B, D, KF), dtype=np.float32)
    livef = n_[None, :] <= L[:, None]
    ga[:, :, :K] = np.where(livef[:, :, None], gfall[:, :D, :], 0.0)
    holdon = n_[None, :] >= (L[:, None] + 1)
    ga[:, :, HOLD] = np.where(holdon, 1.0, 0.0)

    def x_rows(j):
        bjm1, bj = BND[j - 1], BND[j]
        act = L > bj
        tau = bjm1 + n_
        g = np.where(
            act[:, None, None], gfall[:, np.clip(tau - 1, 0, T - 1), :], 0.0
        ).astype(np.float32)
        return g, act

    def y_rows(j):
        bjm1, bj = BND[j - 1], BND[j]
        act = L >= bjm1 + 1
        sstar = np.maximum(bj - L, 0)
        g = np.zeros((B, D, KF), dtype=np.float32)
        tau = bj - n_
        valid = (
            (n_[None, :] >= np.maximum(sstar, 1)[:, None])
            & (n_[None, :] <= D - 1)
            & act[:, None]
        )
        gv = gball[:, np.clip(tau - 1, 0, T - 1), :]
        g[:, :, :K] = np.where(valid[:, :, None], gv, 0.0)
        g[act, D - 1, :K] = 1.0
        srcon = (n_[None, :] < sstar[:, None]) & act[:, None]
        g[:, :, SRCL] = np.where(srcon, 1.0, 0.0)
        u0 = np.zeros((B, KF), dtype=np.float32)
        inj = act & (sstar == 0)
        u0[inj, :K] = gball[inj, bj - 1, :] * estop32[None, :]
        u0[act & (sstar > 0), SRCL] = 1.0
        return g, u0

    def b_rows():
        bm1 = BND[M - 1]
        act = L >= bm1 + 1
        sstar = bm1 + 1 + D - L
        g = np.zeros((B, D, KF), dtype=np.float32)
        tau = bm1 + 1 + D - n_
        valid = (
            (n_[None, :] >= sstar[:, None]) & (n_[None, :] <= D - 1) & act[:, None]
        )
        gv = gball[:, np.clip(tau - 1, 0, T - 1), :]
        g[:, :, :K] = np.where(valid[:, :, None], gv, 0.0)
        g[act, D - 1, :K] = 1.0
        srcon = (n_[None, :] < sstar[:, None]) & act[:, None]
        g[:, :, SRCL] = np.where(srcon, 1.0, 0.0)
        u0 = np.zeros((B, KF), dtype=np.float32)
        u0[act, SRCL] = 1.0
        return g, u0

    xs = {}
    ys = {}
    acts = {}
    for j in range(2, M):
        xs[j], acts[j] = x_rows(j)
        ys[j] = y_rows(j)
    gb_, u0b = b_rows()

    def pack(gf, gbk):
        g = np.zeros((B, D, K2), dtype=np.float32)
        g[:, :, : gf.shape[2]] = gf
        g[:, :, KF : KF + gbk.shape[2]] = gbk
        return g

    P = [pack(ga, ys[2][0])]
    U = []
    u = np.zeros((B, K2), dtype=np.float32)
    u[:, START] = 1.0
    u[:, KF:] = ys[2][1]
    U.append(u)
    for j in range(2, M - 1):
        P.append(pack(xs[j], ys[j + 1][0]))
        u = np.zeros((B, K2), dtype=np.float32)
        u[acts[j], :K] = 1.0
        u[:, KF:] = ys[j + 1][1]
        U.append(u)
    P.append(pack(xs[M - 1], gb_))
    u = np.zeros((B, K2), dtype=np.float32)
    u[acts[M - 1], :K] = 1.0
    u[:, KF:] = u0b
    U.append(u)

    per_core = []
    for cix in range(NCORES):
        sl = slice(cix * BC, (cix + 1) * BC)
        gA = np.concatenate(
            [P[k][sl].transpose(2, 1, 0) for k in range(NG)], axis=2
        ).astype(ml_dtypes.float8_e4m3fn)
        gB = np.concatenate(
            [P[k][sl].transpose(2, 1, 0) for k in range(NG, NP)], axis=2
        ).astype(ml_dtypes.float8_e4m3fn)
        w0 = np.concatenate([U[k][sl].T for k in range(NP)], axis=1).astype(
            ml_dtypes.bfloat16
        )
        per_core.append(
            {
                "emisA": np.ascontiguousarray(gA),
                "emisB": np.ascontiguousarray(gB),
                "w0": np.ascontiguousarray(w0),
            }
        )

    S_ = np.zeros((K2, K2), dtype=np.float32)
    S_[:K, :K] = np.exp(trans).T
    S_[:K, HOLD] = estop32
    S_[HOLD, HOLD] = 1.0
    S_[KF : KF + K, KF : KF + K] = np.exp(trans)
    S_[KF + SRCL, KF : KF + K] = estop32
    S_[KF + SRCL, KF + SRCL] = 1.0
    etil = S_.astype(ml_dtypes.bfloat16)

    ar = np.arange(B)
    ctx = {
        "seq_len": L,
        "estop": estop,
        "C_at_L": Ccum[ar, L - 1],
        "Cal": Ccum[:, BND[1] - 1],
        "Cx": {j: Ccum[:, BND[j] - 1] - Ccum[:, BND[j - 1] - 1] for j in range(2, M)},
        "CyL": {j: Ccumb[ar, L - 1] - Ccumb[:, BND[j - 1] - 1] for j in range(2, M)},
        "Cb": Ccumb[ar, L - 1] - Ccumb[:, BND[M - 1] - 1],
    }
    return per_core, etil, ctx


def _combine(packs, ctx):
    """packs: list of NP arrays [K2, B] f64; returns per-batch scores."""
    L = ctx["seq_len"]
    estop = ctx["estop"]
    alpha = packs[0][:KF, :]
    xv = {}
    yv = {2: packs[0][KF : KF + K, :]}
    for j in range(2, M):
        xv[j] = packs[j - 1][:K, :]
    for j in range(2, M - 1):
        yv[j + 1] = packs[j - 1][KF : KF + K, :]
    bv = packs[NP - 1][KF : KF + K, :]

    d = {2: (yv[2] * alpha[:K, :]).sum(0)}
    n = {2: yv[2].sum(0)}
    for j in range(3, M):
        d[j] = (yv[j] * xv[j - 1]).sum(0)
        n[j] = yv[j].sum(0)
    d_b = (bv * xv[M - 1]).sum(0)

    scores = np.zeros(B)
    J1 = L <= BND[1] - 1
    scores[J1] = np.log(alpha[HOLD, J1]) + ctx["C_at_L"][J1]
    JM_ = L == BND[1]
    if JM_.any():
        dm = (alpha[:K, :] * estop[:, None]).sum(0)
        scores[JM_] = np.log(dm[JM_]) + ctx["Cal"][JM_]
    with np.errstate(divide="ignore", invalid="ignore"):
        for J in range(2, M):
            msk = (L > BND[J - 1]) & (L <= BND[J])
            sc = np.log(d[J]) + ctx["Cal"] + ctx["CyL"][J]
            for j in range(2, J):
                sc = sc + np.log(d[j]) - np.log(n[j]) + ctx["Cx"][j]
            scores[msk] = sc[msk]
        mskM = L > BND[M - 1]
        sc = np.log(d_b) + ctx["Cal"] + ctx["Cb"]
        for j in range(2, M):
            sc = sc + np.log(d[j]) - np.log(n[j]) + ctx["Cx"][j]
        scores[mskM] = sc[mskM]
    return scores


def _gold_score(feats, tags, seq_len, trans):
    feats = np.asarray(feats, dtype=np.float32)
    tags = np.asarray(tags, dtype=np.int64)
    seq_len = np.asarray(seq_len, dtype=np.int64)
    trans = np.asarray(trans, dtype=np.float32)
    tags_ext = np.concatenate(
        [np.full((B, 1), START, dtype=np.int64), tags], axis=1
    )
    trans_sc = trans[tags_ext[:, 1:], tags_ext[:, :-1]]
    emit_sc = np.take_along_axis(feats, tags_ext[:, 1:, None], axis=2)[..., 0]
    mask = np.arange(T)[None, :] < seq_len[:, None]
    last_tag = np.take_along_axis(tags_ext, seq_len[:, None], axis=1)[:, 0]
    gold = (
        np.where(mask, trans_sc + emit_sc, 0.0).sum(1, dtype=np.float64)
        + trans[STOP, last_tag]
    )
    return gold  # [B] f64


def kernel(feats, tags, seq_len, transitions):
    feats = np.asarray(feats)
    per_core, etil, ctx = _host_prep(feats, seq_len, transitions)
    nc = _build_module()
    in_maps = [{"etil": etil, **per_core[c]} for c in range(NCORES)]
    res = run_bass_kernel_spmd(nc, in_maps, list(range(NCORES)))
    outs = [np.asarray(res.results[c]["uout"]).astype(np.float64) for c in range(NCORES)]
    packs = [
        np.concatenate([o[:, k * BC : (k + 1) * BC] for o in outs], axis=1)
        for k in range(NP)
    ]
    scores = _combine(packs, ctx)
    gold = _gold_score(feats, tags, seq_len, transitions)
    loss = np.mean(scores - gold)
    return np.float32(loss)


# revision 27
# speedup vs baseline: 1.0351x; 1.0351x over previous
"""CRF negative log-likelihood loss on 8 Trainium2 NeuronCores.

Strategy: data-parallel over batch (64 sequences per core) with an
M=9-segment split of each sequence cutting the serial scan depth to 114
device slots. Boundaries BND[j] = j*114 (last segment 912..1024).

  alpha: exact forward chain over seg1 (48 states + hold for short L)
  x_j = T_j . 1         (fwd-seeded through middle segment j)
  y_j = T_j^T . e_stop  (bwd-seeded; doubles as the exact tail for batches
                         whose L falls inside segment j via a src state
                         injected at tau=L)
  b = A^T G_{913} beta_{913} (exact backward chain over the last segment)

Products of >=114 positive transfer matrices are numerically rank-1
(Birkhoff contraction), so T_j w ~ x_j (y_j.w)/(y_j.1) and the partition
function composes from host-side dots telescoping across segments:
  Z ~ (b.x_{M-1}) prod_j [(y_j.x_{j-1})/(y_j.1)] (y_2.alpha)
truncated at the segment containing L.

Packing: the 2(M-1) half-chains form M-1=8 packs sharing one block [98,98]
bf16 stationary: P1=[alpha|y2], Pj=[x_j|y_{j+1}], P8=[x_8|b]. Packs are
grouped into TWO super-chains of 4 packs, each a [98, 256] datapath: one
PE matmul + one wide DVE Hadamard per slot (the wide TT amortizes the
fixed 125ns PSUM-access cost over 256 columns). The two super-chains are
independent and interleave on the engines, hiding cross-engine sync
latency. All chains run in the exponential domain (bf16, fp32 PSUM) with
host-precomputed per-(batch,step) shifts; the gold path score is a cheap
host gather.
"""
import numpy as np
import ml_dtypes
from contextlib import ExitStack

import concourse.bacc as bacc
import concourse.bass as bass
import concourse.tile as tile
from concourse import mybir
from concourse.bass_utils import run_bass_kernel_spmd

B, T, K = 512, 1024, 48
START, STOP = 46, 47
NEG = -10000.0
HOLD = 48
SRCL = 48
KF = 49
K2 = 98
NCORES = 8
BC = B // NCORES    # 64
M = 9               # segments
NP = M - 1          # packs (8)
NG = NP // 2        # packs per super-chain (4)
W = NG * BC         # super-chain width (256)
D = -(-1025 // M)   # 114 device slots
BND = [j * D for j in range(M)] + [1024]
CH = 8              # slots per emission chunk (114 = 3*38)

_nc_cache = {}


def _build_module(d_slots=D, ch=CH):
    key = ("nc", d_slots, ch)
    if key in _nc_cache:
        return _nc_cache[key]
    nc = bacc.Bacc(
        "TRN2",
        target_bir_lowering=False,
        debug=False,
        enable_asserts=False,
        num_devices=NCORES,
    )
    f32 = mybir.dt.float32
    bf16 = mybir.dt.bfloat16
    fp8 = mybir.dt.float8e4
    e_dram = nc.dram_tensor("etil", [K2, K2], bf16, kind="ExternalInput").ap()
    gA_dram = nc.dram_tensor("emisA", [K2, d_slots, W], fp8, kind="ExternalInput").ap()
    gB_dram = nc.dram_tensor("emisB", [K2, d_slots, W], fp8, kind="ExternalInput").ap()
    w0_dram = nc.dram_tensor("w0", [K2, 2 * W], bf16, kind="ExternalInput").ap()
    o_dram = nc.dram_tensor("uout", [K2, 2 * W], bf16, kind="ExternalOutput").ap()

    with tile.TileContext(nc) as tc:
        with ExitStack() as ctx:
            const = ctx.enter_context(tc.tile_pool(name="const", bufs=1))
            wpool = ctx.enter_context(tc.tile_pool(name="wp", bufs=4))
            gexp_p = ctx.enter_context(tc.tile_pool(name="gexp", bufs=3))
            psum_p = ctx.enter_context(tc.tile_pool(name="ps", bufs=2, space="PSUM"))

            etile = const.tile([K2, K2], bf16)
            nc.sync.dma_start(out=etile, in_=e_dram)

            wA = const.tile([K2, W], bf16)
            nc.sync.dma_start(out=wA, in_=w0_dram[:, 0:W])
            wB = const.tile([K2, W], bf16)
            nc.scalar.dma_start(out=wB, in_=w0_dram[:, W : 2 * W])

            outT = const.tile([K2, 2 * W], bf16)
            outA = outT[:, 0:W]
            outB = outT[:, W : 2 * W]

            nstep = 0
            sched = [2, 3, 5, 8] + [ch] * 100
            while nstep < d_slots:
                ns = min(sched.pop(0), d_slots - nstep)
                geA = gexp_p.tile([K2, ch, W], fp8, tag="geA")
                nc.sync.dma_start(
                    out=geA[:, :ns, :], in_=gA_dram[:, nstep : nstep + ns, :]
                )
                geB = gexp_p.tile([K2, ch, W], fp8, tag="geB")
                nc.scalar.dma_start(
                    out=geB[:, :ns, :], in_=gB_dram[:, nstep : nstep + ns, :]
                )
                for s in range(ns):
                    last = nstep + s == d_slots - 1
                    psA = psum_p.tile([K2, W], f32, tag="psA")
                    for c0 in range(0, W, 512):
                        c1 = min(c0 + 512, W)
                        nc.tensor.matmul(
                            psA[:, c0:c1], etile, wA[:, c0:c1], start=True, stop=True
                        )
                    psB = psum_p.tile([K2, W], f32, tag="psB")
                    for c0 in range(0, W, 512):
                        c1 = min(c0 + 512, W)
                        nc.tensor.matmul(
                            psB[:, c0:c1], etile, wB[:, c0:c1], start=True, stop=True
                        )
                    if last:
                        nc.vector.tensor_mul(outA, psA, geA[:, s, :])
                        nc.vector.tensor_mul(outB, psB, geB[:, s, :])
                    else:
                        wAn = wpool.tile([K2, W], bf16, tag="wA")
                        nc.vector.tensor_mul(wAn, psA, geA[:, s, :])
                        wA = wAn
                        wBn = wpool.tile([K2, W], bf16, tag="wB")
                        nc.vector.tensor_mul(wBn, psB, geB[:, s, :])
                        wB = wBn
                nstep += ns
            nc.sync.dma_start(out=o_dram, in_=outT)

    nc.compile()
    _nc_cache[key] = nc
    return nc


def _host_prep(feats, seq_len, trans):
    feats = np.ascontiguousarray(feats, dtype=np.float32)
    seq_len = np.asarray(seq_len, dtype=np.int64)
    trans = np.asarray(trans, dtype=np.float32)
    L = seq_len

    mx = feats.max(axis=2)
    E64 = np.exp(trans.astype(np.float64)).T

    drift = []
    for b in range(6):
        fv = np.full(K, NEG, dtype=np.float64)
        fv[START] = 0.0
        Lb = int(L[b])
        for t in range(min(Lb, 256)):
            m = fv.max()
            wv = np.exp(fv - m)
            with np.errstate(divide="ignore"):
                fv = np.log(E64.T @ wv) + m + feats[b, t]
            drift.append((fv.max() - m) - mx[b, t])
    mu = float(np.mean(drift))

    driftb = []
    nb = 0
    for b in range(B):
        Lb = int(L[b])
        if Lb < 700:
            continue
        nb += 1
        if nb > 6:
            break
        bv = trans[STOP, :].astype(np.float64).copy()
        for t in range(Lb, Lb - 200, -1):
            m = bv.max()
            wv = np.exp(bv - m)
            gv = np.exp(feats[b, t - 1].astype(np.float64))
            with np.errstate(divide="ignore"):
                bv = np.log(E64 @ (gv * wv)) + m
            driftb.append((bv.max() - m) - mx[b, t - 1])
    mub = float(np.mean(driftb)) if driftb else mu

    c = mx + mu
    cb = mx + mub
    Ccum = np.cumsum(c, axis=1, dtype=np.float64)
    Ccumb = np.cumsum(cb, axis=1, dtype=np.float64)
    estop = np.exp(trans[STOP, :K].astype(np.float64))
    estop32 = estop.astype(np.float32)

    n_ = np.arange(1, D + 1)
    gfall = np.exp(feats - c[:, :, None])
    gball = np.exp(feats - cb[:, :, None])

    ga = np.zeros((B, D, KF), dtype=np.float32)
    livef = n_[None, :] <= L[:, None]
    ga[:, :, :K] = np.where(livef[:, :, None], gfall[:, :D, :], 0.0)
    holdon = n_[None, :] >= (L[:, None] + 1)
    ga[:, :, HOLD] = np.where(holdon, 1.0, 0.0)

    def x_rows(j):
        bjm1, bj = BND[j - 1], BND[j]
        act = L > bj
        tau = bjm1 + n_
        g = np.where(
            act[:, None, None], gfall[:, np.clip(tau - 1, 0, T - 1), :], 0.0
        ).astype(np.float32)
        return g, act

    def y_rows(j):
        bjm1, bj = BND[j - 1], BND[j]
        act = L >= bjm1 + 1
        sstar = np.maximum(bj - L, 0)
        g = np.zeros((B, D, KF), dtype=np.float32)
        tau = bj - n_
        valid = (
            (n_[None, :] >= np.maximum(sstar, 1)[:, None])
            & (n_[None, :] <= D - 1)
            & act[:, None]
        )
        gv = gball[:, np.clip(tau - 1, 0, T - 1), :]
        g[:, :, :K] = np.where(valid[:, :, None], gv, 0.0)
        g[act, D - 1, :K] = 1.0
        srcon = (n_[None, :] < sstar[:, None]) & act[:, None]
        g[:, :, SRCL] = np.where(srcon, 1.0, 0.0)
        u0 = np.zeros((B, KF), dtype=np.float32)
        inj = act & (sstar == 0)
        u0[inj, :K] = gball[inj, bj - 1, :] * estop32[None, :]
        u0[act & (sstar > 0), SRCL] = 1.0
        return g, u0

    def b_rows():
        bm1 = BND[M - 1]
        act = L >= bm1 + 1
        sstar = bm1 + 1 + D - L
        g = np.zeros((B, D, KF), dtype=np.float32)
        tau = bm1 + 1 + D - n_
        valid = (
            (n_[None, :] >= sstar[:, None]) & (n_[None, :] <= D - 1) & act[:, None]
        )
        gv = gball[:, np.clip(tau - 1, 0, T - 1), :]
        g[:, :, :K] = np.where(valid[:, :, None], gv, 0.0)
        g[act, D - 1, :K] = 1.0
        srcon = (n_[None, :] < sstar[:, None]) & act[:, None]
        g[:, :, SRCL] = np.where(srcon, 1.0, 0.0)
        u0 = np.zeros((B, KF), dtype=np.float32)
        u0[act, SRCL] = 1.0
        return g, u0

    xs = {}
    ys = {}
    acts = {}
    for j in range(2, M):
        xs[j], acts[j] = x_rows(j)
        ys[j] = y_rows(j)
    gb_, u0b = b_rows()

    def pack(gf, gbk):
        g = np.zeros((B, D, K2), dtype=np.float32)
        g[:, :, : gf.shape[2]] = gf
        g[:, :, KF : KF + gbk.shape[2]] = gbk
        return g

    P = [pack(ga, ys[2][0])]
    U = []
    u = np.zeros((B, K2), dtype=np.float32)
    u[:, START] = 1.0
    u[:, KF:] = ys[2][1]
    U.append(u)
    for j in range(2, M - 1):
        P.append(pack(xs[j], ys[j + 1][0]))
        u = np.zeros((B, K2), dtype=np.float32)
        u[acts[j], :K] = 1.0
        u[:, KF:] = ys[j + 1][1]
        U.append(u)
    P.append(pack(xs[M - 1], gb_))
    u = np.zeros((B, K2), dtype=np.float32)
    u[acts[M - 1], :K] = 1.0
    u[:, KF:] = u0b
    U.append(u)

    per_core = []
    for cix in range(NCORES):
        sl = slice(cix * BC, (cix + 1) * BC)
        gA = np.concatenate(
            [P[k][sl].transpose(2, 1, 0) for k in range(NG)], axis=2
        ).astype(ml_dtypes.float8_e4m3fn)
        gB = np.concatenate(
            [P[k][sl].transpose(2, 1, 0) for k in range(NG, NP)], axis=2
        ).astype(ml_dtypes.float8_e4m3fn)
        w0 = np.concatenate([U[k][sl].T for k in range(NP)], axis=1).astype(
            ml_dtypes.bfloat16
        )
        per_core.append(
            {
                "emisA": np.ascontiguousarray(gA),
                "emisB": np.ascontiguousarray(gB),
                "w0": np.ascontiguousarray(w0),
            }
        )

    S_ = np.zeros((K2, K2), dtype=np.float32)
    S_[:K, :K] = np.exp(trans).T
    S_[:K, HOLD] = estop32
    S_[HOLD, HOLD] = 1.0
    S_[KF : KF + K, KF : KF + K] = np.exp(trans)
    S_[KF + SRCL, KF : KF + K] = estop32
    S_[KF + SRCL, KF + SRCL] = 1.0
    etil = S_.astype(ml_dtypes.bfloat16)

    ar = np.arange(B)
    ctx = {
        "seq_len": L,
        "estop": estop,
        "C_at_L": Ccum[ar, L - 1],
        "Cal": Ccum[:, BND[1] - 1],
        "Cx": {j: Ccum[:, BND[j] - 1] - Ccum[:, BND[j - 1] - 1] for j in range(2, M)},
        "CyL": {j: Ccumb[ar, L - 1] - Ccumb[:, BND[j - 1] - 1] for j in range(2, M)},
        "Cb": Ccumb[ar, L - 1] - Ccumb[:, BND[M - 1] - 1],
    }
    return per_core, etil, ctx


def _combine(packs, ctx):
    """packs: list of NP arrays [K2, B] f64; returns per-batch scores."""
    L = ctx["seq_len"]
    estop = ctx["estop"]
    alpha = packs[0][:KF, :]
    xv = {}
    yv = {2: packs[0][KF : KF + K, :]}
    for j in range(2, M):
        xv[j] = packs[j - 1][:K, :]
    for j in range(2, M - 1):
        yv[j + 1] = packs[j - 1][KF : KF + K, :]
    bv = packs[NP - 1][KF : KF + K, :]

    d = {2: (yv[2] * alpha[:K, :]).sum(0)}
    n = {2: yv[2].sum(0)}
    for j in range(3, M):
        d[j] = (yv[j] * xv[j - 1]).sum(0)
        n[j] = yv[j].sum(0)
    d_b = (bv * xv[M - 1]).sum(0)

    scores = np.zeros(B)
    J1 = L <= BND[1] - 1
    scores[J1] = np.log(alpha[HOLD, J1]) + ctx["C_at_L"][J1]
    JM_ = L == BND[1]
    if JM_.any():
        dm = (alpha[:K, :] * estop[:, None]).sum(0)
        scores[JM_] = np.log(dm[JM_]) + ctx["Cal"][JM_]
    with np.errstate(divide="ignore", invalid="ignore"):
        for J in range(2, M):
            msk = (L > BND[J - 1]) & (L <= BND[J])
            sc = np.log(d[J]) + ctx["Cal"] + ctx["CyL"][J]
            for j in range(2, J):
                sc = sc + np.log(d[j]) - np.log(n[j]) + ctx["Cx"][j]
            scores[msk] = sc[msk]
        mskM = L > BND[M - 1]
        sc = np.log(d_b) + ctx["Cal"] + ctx["Cb"]
        for j in range(2, M):
            sc = sc + np.log(d[j]) - np.log(n[j]) + ctx["Cx"][j]
        scores[mskM] = sc[mskM]
    return scores


def _gold_score(feats, tags, seq_len, trans):
    feats = np.asarray(feats, dtype=np.float32)
    tags = np.asarray(tags, dtype=np.int64)
    seq_len = np.asarray(seq_len, dtype=np.int64)
    trans = np.asarray(trans, dtype=np.float32)
    tags_ext = np.concatenate(
        [np.full((B, 1), START, dtype=np.int64), tags], axis=1
    )
    trans_sc = trans[tags_ext[:, 1:], tags_ext[:, :-1]]
    emit_sc = np.take_along_axis(feats, tags_ext[:, 1:, None], axis=2)[..., 0]
    mask = np.arange(T)[None, :] < seq_len[:, None]
    last_tag = np.take_along_axis(tags_ext, seq_len[:, None], axis=1)[:, 0]
    gold = (
        np.where(mask, trans_sc + emit_sc, 0.0).sum(1, dtype=np.float64)
        + trans[STOP, last_tag]
    )
    return gold  # [B] f64


def kernel(feats, tags, seq_len, transitions):
    feats = np.asarray(feats)
    per_core, etil, ctx = _host_prep(feats, seq_len, transitions)
    nc = _build_module()
    in_maps = [{"etil": etil, **per_core[c]} for c in range(NCORES)]
    res = run_bass_kernel_spmd(nc, in_maps, list(range(NCORES)))
    outs = [np.asarray(res.results[c]["uout"]).astype(np.float64) for c in range(NCORES)]
    packs = [
        np.concatenate([o[:, k * BC : (k + 1) * BC] for o in outs], axis=1)
        for k in range(NP)
    ]
    scores = _combine(packs, ctx)
    gold = _gold_score(feats, tags, seq_len, transitions)
    loss = np.mean(scores - gold)
    return np.float32(loss)


# revision 28
# speedup vs baseline: 1.5953x; 1.5411x over previous
"""CRF negative log-likelihood loss on 8 Trainium2 NeuronCores.

Strategy: data-parallel over batch (64 sequences per core) with an
M=9-segment split of each sequence cutting the serial scan depth to 114
device slots. Boundaries BND[j] = j*114 (last segment 912..1024).

  alpha: exact forward chain over seg1 (48 states + hold for short L)
  x_j = T_j . 1         (fwd-seeded through middle segment j)
  y_j = T_j^T . e_stop  (bwd-seeded; doubles as the exact tail for batches
                         whose L falls inside segment j via a src state
                         injected at tau=L)
  b = A^T G_{913} beta_{913} (exact backward chain over the last segment)

Products of >=114 positive transfer matrices are numerically rank-1
(Birkhoff contraction), so T_j w ~ x_j (y_j.w)/(y_j.1) and the partition
function composes from host-side dots telescoping across segments:
  Z ~ (b.x_{M-1}) prod_j [(y_j.x_{j-1})/(y_j.1)] (y_2.alpha)
truncated at the segment containing L.

Packing: the 2(M-1) half-chains form M-1=8 packs sharing one block [98,98]
bf16 stationary: P1=[alpha|y2], Pj=[x_j|y_{j+1}], P8=[x_8|b]. Packs are
grouped into TWO super-chains of 4 packs, each a [98, 256] datapath: one
PE matmul + one wide DVE Hadamard per slot (the wide TT amortizes the
fixed 125ns PSUM-access cost over 256 columns). The two super-chains are
independent and interleave on the engines, hiding cross-engine sync
latency. All chains run in the exponential domain (bf16, fp32 PSUM) with
host-precomputed per-(batch,step) shifts; the gold path score is a cheap
host gather.
"""
import numpy as np
import ml_dtypes
from contextlib import ExitStack

import concourse.bacc as bacc
import concourse.bass as bass
import concourse.tile as tile
from concourse import mybir
from concourse.bass_utils import run_bass_kernel_spmd

B, T, K = 512, 1024, 48
START, STOP = 46, 47
NEG = -10000.0
HOLD = 48
SRCL = 48
KF = 49
K2 = 98
NCORES = 8
BC = B // NCORES    # 64
M = 9               # segments
NP = M - 1          # packs
W = 512             # columns per super-chain (compacted active half-chains)
D = -(-1025 // M)   # 114 device slots
BND = [j * D for j in range(M)] + [1024]
CH = 8              # slots per emission chunk (114 = 3*38)

_nc_cache = {}


def _build_module(d_slots=D, ch=CH):
    key = ("nc", d_slots, ch)
    if key in _nc_cache:
        return _nc_cache[key]
    nc = bacc.Bacc(
        "TRN2",
        target_bir_lowering=False,
        debug=False,
        enable_asserts=False,
        num_devices=NCORES,
    )
    f32 = mybir.dt.float32
    bf16 = mybir.dt.bfloat16
    fp8 = mybir.dt.float8e4
    e_dram = nc.dram_tensor("etil", [K2, K2], bf16, kind="ExternalInput").ap()
    gA_dram = nc.dram_tensor("emisA", [K2, d_slots, W], fp8, kind="ExternalInput").ap()
    gB_dram = nc.dram_tensor("emisB", [K2, d_slots, W], fp8, kind="ExternalInput").ap()
    w0_dram = nc.dram_tensor("w0", [K2, 2 * W], bf16, kind="ExternalInput").ap()
    o_dram = nc.dram_tensor("uout", [K2, 2 * W], bf16, kind="ExternalOutput").ap()

    with tile.TileContext(nc) as tc:
        with ExitStack() as ctx:
            const = ctx.enter_context(tc.tile_pool(name="const", bufs=1))
            wpool = ctx.enter_context(tc.tile_pool(name="wp", bufs=4))
            gexp_p = ctx.enter_context(tc.tile_pool(name="gexp", bufs=3))
            psum_p = ctx.enter_context(tc.tile_pool(name="ps", bufs=4, space="PSUM"))

            etile = const.tile([K2, K2], bf16)
            nc.sync.dma_start(out=etile, in_=e_dram)

            wA = const.tile([K2, W], bf16)
            nc.sync.dma_start(out=wA, in_=w0_dram[:, 0:W])
            wB = const.tile([K2, W], bf16)
            nc.scalar.dma_start(out=wB, in_=w0_dram[:, W : 2 * W])

            outT = const.tile([K2, 2 * W], bf16)
            outA = outT[:, 0:W]
            outB = outT[:, W : 2 * W]

            nstep = 0
            sched = [2, 3, 5, 8] + [ch] * 100
            while nstep < d_slots:
                ns = min(sched.pop(0), d_slots - nstep)
                geA = gexp_p.tile([K2, ch, W], fp8, tag="geA")
                nc.sync.dma_start(
                    out=geA[:, :ns, :], in_=gA_dram[:, nstep : nstep + ns, :]
                )
                geB = gexp_p.tile([K2, ch, W], fp8, tag="geB")
                nc.scalar.dma_start(
                    out=geB[:, :ns, :], in_=gB_dram[:, nstep : nstep + ns, :]
                )
                for s in range(ns):
                    last = nstep + s == d_slots - 1
                    psA = psum_p.tile([K2, W], f32, tag="psA")
                    nc.tensor.matmul(psA, etile, wA, start=True, stop=True)
                    psB = psum_p.tile([K2, W], f32, tag="psB")
                    nc.tensor.matmul(psB, etile, wB, start=True, stop=True)
                    if last:
                        nc.vector.tensor_mul(outA, psA, geA[:, s, :])
                        nc.vector.tensor_mul(outB, psB, geB[:, s, :])
                    else:
                        wAn = wpool.tile([K2, W], bf16, tag="wA")
                        nc.vector.tensor_mul(wAn, psA, geA[:, s, :])
                        wA = wAn
                        wBn = wpool.tile([K2, W], bf16, tag="wB")
                        nc.vector.tensor_mul(wBn, psB, geB[:, s, :])
                        wB = wBn
                nstep += ns
            nc.sync.dma_start(out=o_dram, in_=outT)

    nc.compile()
    _nc_cache[key] = nc
    return nc


def _host_prep(feats, seq_len, trans):
    feats = np.ascontiguousarray(feats, dtype=np.float32)
    seq_len = np.asarray(seq_len, dtype=np.int64)
    trans = np.asarray(trans, dtype=np.float32)
    L = seq_len

    mx = feats.max(axis=2)
    E64 = np.exp(trans.astype(np.float64)).T

    drift = []
    for b in range(6):
        fv = np.full(K, NEG, dtype=np.float64)
        fv[START] = 0.0
        Lb = int(L[b])
        for t in range(min(Lb, 256)):
            m = fv.max()
            wv = np.exp(fv - m)
            with np.errstate(divide="ignore"):
                fv = np.log(E64.T @ wv) + m + feats[b, t]
            drift.append((fv.max() - m) - mx[b, t])
    mu = float(np.mean(drift))

    driftb = []
    nb = 0
    for b in range(B):
        Lb = int(L[b])
        if Lb < 700:
            continue
        nb += 1
        if nb > 6:
            break
        bv = trans[STOP, :].astype(np.float64).copy()
        for t in range(Lb, Lb - 200, -1):
            m = bv.max()
            wv = np.exp(bv - m)
            gv = np.exp(feats[b, t - 1].astype(np.float64))
            with np.errstate(divide="ignore"):
                bv = np.log(E64 @ (gv * wv)) + m
            driftb.append((bv.max() - m) - mx[b, t - 1])
    mub = float(np.mean(driftb)) if driftb else mu

    c = mx + mu
    cb = mx + mub
    Ccum = np.cumsum(c, axis=1, dtype=np.float64)
    Ccumb = np.cumsum(cb, axis=1, dtype=np.float64)
    estop = np.exp(trans[STOP, :K].astype(np.float64))
    estop32 = estop.astype(np.float32)

    n_ = np.arange(1, D + 1)
    gfall = np.exp(feats - c[:, :, None])
    gball = np.exp(feats - cb[:, :, None])

    ga = np.zeros((B, D, KF), dtype=np.float32)
    livef = n_[None, :] <= L[:, None]
    ga[:, :, :K] = np.where(livef[:, :, None], gfall[:, :D, :], 0.0)
    holdon = n_[None, :] >= (L[:, None] + 1)
    ga[:, :, HOLD] = np.where(holdon, 1.0, 0.0)

    def x_rows(j):
        bjm1, bj = BND[j - 1], BND[j]
        act = L > bj
        tau = bjm1 + n_
        g = np.where(
            act[:, None, None], gfall[:, np.clip(tau - 1, 0, T - 1), :], 0.0
        ).astype(np.float32)
        return g, act

    def y_rows(j):
        bjm1, bj = BND[j - 1], BND[j]
        act = L >= bjm1 + 1
        sstar = np.maximum(bj - L, 0)
        g = np.zeros((B, D, KF), dtype=np.float32)
        tau = bj - n_
        valid = (
            (n_[None, :] >= np.maximum(sstar, 1)[:, None])
            & (n_[None, :] <= D - 1)
            & act[:, None]
        )
        gv = gball[:, np.clip(tau - 1, 0, T - 1), :]
        g[:, :, :K] = np.where(valid[:, :, None], gv, 0.0)
        g[act, D - 1, :K] = 1.0
        srcon = (n_[None, :] < sstar[:, None]) & act[:, None]
        g[:, :, SRCL] = np.where(srcon, 1.0, 0.0)
        u0 = np.zeros((B, KF), dtype=np.float32)
        inj = act & (sstar == 0)
        u0[inj, :K] = gball[inj, bj - 1, :] * estop32[None, :]
        u0[act & (sstar > 0), SRCL] = 1.0
        return g, u0

    def b_rows():
        bm1 = BND[M - 1]
        act = L >= bm1 + 1
        sstar = bm1 + 1 + D - L
        g = np.zeros((B, D, KF), dtype=np.float32)
        tau = bm1 + 1 + D - n_
        valid = (
            (n_[None, :] >= sstar[:, None]) & (n_[None, :] <= D - 1) & act[:, None]
        )
        gv = gball[:, np.clip(tau - 1, 0, T - 1), :]
        g[:, :, :K] = np.where(valid[:, :, None], gv, 0.0)
        g[act, D - 1, :K] = 1.0
        srcon = (n_[None, :] < sstar[:, None]) & act[:, None]
        g[:, :, SRCL] = np.where(srcon, 1.0, 0.0)
        u0 = np.zeros((B, KF), dtype=np.float32)
        u0[act, SRCL] = 1.0
        return g, u0

    xs = {}
    ys = {}
    acts = {}
    for j in range(2, M):
        xs[j], acts[j] = x_rows(j)
        ys[j] = y_rows(j)
    gb_, u0b = b_rows()

    def pack(gf, gbk):
        g = np.zeros((B, D, K2), dtype=np.float32)
        g[:, :, : gf.shape[2]] = gf
        g[:, :, KF : KF + gbk.shape[2]] = gbk
        return g

    # compacted column assignment: each column hosts one active fwd
    # half-chain (rows 0:49) and one active bwd half-chain (rows 49:98)
    e_start = np.zeros(KF, dtype=np.float32)
    e_start[START] = 1.0
    ones48 = np.zeros(KF, dtype=np.float32)
    ones48[:K] = 1.0
    WTOT = 2 * W
    per_core = []
    maps = []
    for cix in range(NCORES):
        bsl = range(cix * BC, (cix + 1) * BC)
        emis = np.zeros((K2, D, WTOT), dtype=np.float32)
        w0 = np.zeros((K2, WTOT), dtype=np.float32)
        fmap = []
        bmap = []
        for b in bsl:
            i = len(fmap)
            emis[:KF, :, i] = ga[b].T
            w0[:KF, i] = e_start
            fmap.append(("a", 0, b))
        for j in range(2, M):
            for b in bsl:
                if L[b] > BND[j]:
                    i = len(fmap)
                    emis[:K, :, i] = xs[j][b].T
                    w0[:KF, i] = ones48
                    fmap.append(("x", j, b))
        for j in range(2, M):
            for b in bsl:
                if L[b] >= BND[j - 1] + 1:
                    i = len(bmap)
                    emis[KF:, :, i] = ys[j][0][b].T
                    w0[KF:, i] = ys[j][1][b]
                    bmap.append(("y", j, b))
        for b in bsl:
            if L[b] > BND[M - 1]:
                i = len(bmap)
                emis[KF:, :, i] = gb_[b].T
                w0[KF:, i] = u0b[b]
                bmap.append(("b", 0, b))
        assert len(fmap) <= WTOT and len(bmap) <= WTOT, (len(fmap), len(bmap))
        emis8 = emis.astype(ml_dtypes.float8_e4m3fn)
        per_core.append(
            {
                "emisA": np.ascontiguousarray(emis8[:, :, 0:W]),
                "emisB": np.ascontiguousarray(emis8[:, :, W:WTOT]),
                "w0": np.ascontiguousarray(w0.astype(ml_dtypes.bfloat16)),
            }
        )
        maps.append((fmap, bmap))

    S_ = np.zeros((K2, K2), dtype=np.float32)
    S_[:K, :K] = np.exp(trans).T
    S_[:K, HOLD] = estop32
    S_[HOLD, HOLD] = 1.0
    S_[KF : KF + K, KF : KF + K] = np.exp(trans)
    S_[KF + SRCL, KF : KF + K] = estop32
    S_[KF + SRCL, KF + SRCL] = 1.0
    etil = S_.astype(ml_dtypes.bfloat16)

    ar = np.arange(B)
    ctx = {
        "seq_len": L,
        "estop": estop,
        "C_at_L": Ccum[ar, L - 1],
        "Cal": Ccum[:, BND[1] - 1],
        "Cx": {j: Ccum[:, BND[j] - 1] - Ccum[:, BND[j - 1] - 1] for j in range(2, M)},
        "CyL": {j: Ccumb[ar, L - 1] - Ccumb[:, BND[j - 1] - 1] for j in range(2, M)},
        "Cb": Ccumb[ar, L - 1] - Ccumb[:, BND[M - 1] - 1],
        "maps": maps,
    }
    return per_core, etil, ctx


def _combine(parts, ctx):
    """parts: (alpha [KF,B], xv {j:[K,B]}, yv {j:[K,B]}, bv [K,B]) f64."""
    L = ctx["seq_len"]
    estop = ctx["estop"]
    alpha, xv, yv, bv = parts

    d = {2: (yv[2] * alpha[:K, :]).sum(0)}
    n = {2: yv[2].sum(0)}
    for j in range(3, M):
        d[j] = (yv[j] * xv[j - 1]).sum(0)
        n[j] = yv[j].sum(0)
    d_b = (bv * xv[M - 1]).sum(0)

    scores = np.zeros(B)
    J1 = L <= BND[1] - 1
    scores[J1] = np.log(alpha[HOLD, J1]) + ctx["C_at_L"][J1]
    JM_ = L == BND[1]
    if JM_.any():
        dm = (alpha[:K, :] * estop[:, None]).sum(0)
        scores[JM_] = np.log(dm[JM_]) + ctx["Cal"][JM_]
    with np.errstate(divide="ignore", invalid="ignore"):
        for J in range(2, M):
            msk = (L > BND[J - 1]) & (L <= BND[J])
            sc = np.log(d[J]) + ctx["Cal"] + ctx["CyL"][J]
            for j in range(2, J):
                sc = sc + np.log(d[j]) - np.log(n[j]) + ctx["Cx"][j]
            scores[msk] = sc[msk]
        mskM = L > BND[M - 1]
        sc = np.log(d_b) + ctx["Cal"] + ctx["Cb"]
        for j in range(2, M):
            sc = sc + np.log(d[j]) - np.log(n[j]) + ctx["Cx"][j]
        scores[mskM] = sc[mskM]
    return scores


def _gold_score(feats, tags, seq_len, trans):
    feats = np.asarray(feats, dtype=np.float32)
    tags = np.asarray(tags, dtype=np.int64)
    seq_len = np.asarray(seq_len, dtype=np.int64)
    trans = np.asarray(trans, dtype=np.float32)
    tags_ext = np.concatenate(
        [np.full((B, 1), START, dtype=np.int64), tags], axis=1
    )
    trans_sc = trans[tags_ext[:, 1:], tags_ext[:, :-1]]
    emit_sc = np.take_along_axis(feats, tags_ext[:, 1:, None], axis=2)[..., 0]
    mask = np.arange(T)[None, :] < seq_len[:, None]
    last_tag = np.take_along_axis(tags_ext, seq_len[:, None], axis=1)[:, 0]
    gold = (
        np.where(mask, trans_sc + emit_sc, 0.0).sum(1, dtype=np.float64)
        + trans[STOP, last_tag]
    )
    return gold  # [B] f64


def kernel(feats, tags, seq_len, transitions):
    feats = np.asarray(feats)
    per_core, etil, ctx = _host_prep(feats, seq_len, transitions)
    nc = _build_module()
    in_maps = [{"etil": etil, **per_core[c]} for c in range(NCORES)]
    res = run_bass_kernel_spmd(nc, in_maps, list(range(NCORES)))
    outs = [np.asarray(res.results[c]["uout"]).astype(np.float64) for c in range(NCORES)]
    alpha = np.zeros((KF, B))
    xv = {j: np.zeros((K, B)) for j in range(2, M)}
    yv = {j: np.zeros((K, B)) for j in range(2, M)}
    bvv = np.zeros((K, B))
    for c in range(NCORES):
        fmap, bmap = ctx["maps"][c]
        o = outs[c]
        for i, (kind, j, b) in enumerate(fmap):
            if kind == "a":
                alpha[:, b] = o[:KF, i]
            else:
                xv[j][:, b] = o[:K, i]
        for i, (kind, j, b) in enumerate(bmap):
            if kind == "y":
                yv[j][:, b] = o[KF : KF + K, i]
            else:
                bvv[:, b] = o[KF : KF + K, i]
    scores = _combine((alpha, xv, yv, bvv), ctx)
    gold = _gold_score(feats, tags, seq_len, transitions)
    loss = np.mean(scores - gold)
    return np.float32(loss)


# revision 29
# speedup vs baseline: 1.6240x; 1.0180x over previous
"""CRF negative log-likelihood loss on 8 Trainium2 NeuronCores.

Strategy: data-parallel over batch (64 sequences per core) with an
M=9-segment split of each sequence cutting the serial scan depth to 114
device slots. Boundaries BND[j] = j*114 (last segment 912..1024).

  alpha: exact forward chain over seg1 (48 states + hold for short L)
  x_j = T_j . 1         (fwd-seeded through middle segment j)
  y_j = T_j^T . e_stop  (bwd-seeded; doubles as the exact tail for batches
                         whose L falls inside segment j via a src state
                         injected at tau=L)
  b = A^T G_{913} beta_{913} (exact backward chain over the last segment)

Products of >=114 positive transfer matrices are numerically rank-1
(Birkhoff contraction), so T_j w ~ x_j (y_j.w)/(y_j.1) and the partition
function composes from host-side dots telescoping across segments:
  Z ~ (b.x_{M-1}) prod_j [(y_j.x_{j-1})/(y_j.1)] (y_2.alpha)
truncated at the segment containing L.

Packing: the 2(M-1) half-chains form M-1=8 packs sharing one block [98,98]
bf16 stationary: P1=[alpha|y2], Pj=[x_j|y_{j+1}], P8=[x_8|b]. Packs are
grouped into TWO super-chains of 4 packs, each a [98, 256] datapath: one
PE matmul + one wide DVE Hadamard per slot (the wide TT amortizes the
fixed 125ns PSUM-access cost over 256 columns). The two super-chains are
independent and interleave on the engines, hiding cross-engine sync
latency. All chains run in the exponential domain (bf16, fp32 PSUM) with
host-precomputed per-(batch,step) shifts; the gold path score is a cheap
host gather.
"""
import numpy as np
import ml_dtypes
from contextlib import ExitStack

import concourse.bacc as bacc
import concourse.bass as bass
import concourse.tile as tile
from concourse import mybir
from concourse.bass_utils import run_bass_kernel_spmd

B, T, K = 512, 1024, 48
START, STOP = 46, 47
NEG = -10000.0
HOLD = 48
SRCL = 48
KF = 49
K2 = 98
NCORES = 8
BC = B // NCORES    # 64
M = 9               # segments
NP = M - 1          # packs
W = 494             # columns per super-chain (compacted active half-chains)
D = -(-1025 // M)   # 114 device slots
BND = [j * D for j in range(M)] + [1024]
CH = 8              # slots per emission chunk (114 = 3*38)

_nc_cache = {}


def _build_module(d_slots=D, ch=CH):
    key = ("nc", d_slots, ch)
    if key in _nc_cache:
        return _nc_cache[key]
    nc = bacc.Bacc(
        "TRN2",
        target_bir_lowering=False,
        debug=False,
        enable_asserts=False,
        num_devices=NCORES,
    )
    f32 = mybir.dt.float32
    bf16 = mybir.dt.bfloat16
    fp8 = mybir.dt.float8e4
    e_dram = nc.dram_tensor("etil", [K2, K2], bf16, kind="ExternalInput").ap()
    gA_dram = nc.dram_tensor("emisA", [K2, d_slots, W], fp8, kind="ExternalInput").ap()
    gB_dram = nc.dram_tensor("emisB", [K2, d_slots, W], fp8, kind="ExternalInput").ap()
    w0_dram = nc.dram_tensor("w0", [K2, 2 * W], bf16, kind="ExternalInput").ap()
    o_dram = nc.dram_tensor("uout", [K2, 2 * W], bf16, kind="ExternalOutput").ap()

    with tile.TileContext(nc) as tc:
        with ExitStack() as ctx:
            const = ctx.enter_context(tc.tile_pool(name="const", bufs=1))
            wpool = ctx.enter_context(tc.tile_pool(name="wp", bufs=4))
            gexp_p = ctx.enter_context(tc.tile_pool(name="gexp", bufs=3))
            psum_p = ctx.enter_context(tc.tile_pool(name="ps", bufs=4, space="PSUM"))

            etile = const.tile([K2, K2], bf16)
            nc.sync.dma_start(out=etile, in_=e_dram)

            wA = const.tile([K2, W], bf16)
            nc.sync.dma_start(out=wA, in_=w0_dram[:, 0:W])
            wB = const.tile([K2, W], bf16)
            nc.scalar.dma_start(out=wB, in_=w0_dram[:, W : 2 * W])

            outT = const.tile([K2, 2 * W], bf16)
            outA = outT[:, 0:W]
            outB = outT[:, W : 2 * W]

            nstep = 0
            sched = [2, 3, 5, 8] + [ch] * 100
            while nstep < d_slots:
                ns = min(sched.pop(0), d_slots - nstep)
                geA = gexp_p.tile([K2, ch, W], fp8, tag="geA")
                nc.sync.dma_start(
                    out=geA[:, :ns, :], in_=gA_dram[:, nstep : nstep + ns, :]
                )
                geB = gexp_p.tile([K2, ch, W], fp8, tag="geB")
                nc.scalar.dma_start(
                    out=geB[:, :ns, :], in_=gB_dram[:, nstep : nstep + ns, :]
                )
                for s in range(ns):
                    last = nstep + s == d_slots - 1
                    psA = psum_p.tile([K2, W], f32, tag="psA")
                    nc.tensor.matmul(psA, etile, wA, start=True, stop=True)
                    psB = psum_p.tile([K2, W], f32, tag="psB")
                    nc.tensor.matmul(psB, etile, wB, start=True, stop=True)
                    if last:
                        nc.vector.tensor_mul(outA, psA, geA[:, s, :])
                        nc.vector.tensor_mul(outB, psB, geB[:, s, :])
                    else:
                        wAn = wpool.tile([K2, W], bf16, tag="wA")
                        nc.vector.tensor_mul(wAn, psA, geA[:, s, :])
                        wA = wAn
                        wBn = wpool.tile([K2, W], bf16, tag="wB")
                        nc.vector.tensor_mul(wBn, psB, geB[:, s, :])
                        wB = wBn
                nstep += ns
            nc.sync.dma_start(out=o_dram, in_=outT)

    nc.compile()
    _nc_cache[key] = nc
    return nc


def _host_prep(feats, seq_len, trans):
    feats = np.ascontiguousarray(feats, dtype=np.float32)
    seq_len = np.asarray(seq_len, dtype=np.int64)
    trans = np.asarray(trans, dtype=np.float32)
    L = seq_len

    mx = feats.max(axis=2)
    E64 = np.exp(trans.astype(np.float64)).T

    drift = []
    for b in range(6):
        fv = np.full(K, NEG, dtype=np.float64)
        fv[START] = 0.0
        Lb = int(L[b])
        for t in range(min(Lb, 256)):
            m = fv.max()
            wv = np.exp(fv - m)
            with np.errstate(divide="ignore"):
                fv = np.log(E64.T @ wv) + m + feats[b, t]
            drift.append((fv.max() - m) - mx[b, t])
    mu = float(np.mean(drift))

    driftb = []
    nb = 0
    for b in range(B):
        Lb = int(L[b])
        if Lb < 700:
            continue
        nb += 1
        if nb > 6:
            break
        bv = trans[STOP, :].astype(np.float64).copy()
        for t in range(Lb, Lb - 200, -1):
            m = bv.max()
            wv = np.exp(bv - m)
            gv = np.exp(feats[b, t - 1].astype(np.float64))
            with np.errstate(divide="ignore"):
                bv = np.log(E64 @ (gv * wv)) + m
            driftb.append((bv.max() - m) - mx[b, t - 1])
    mub = float(np.mean(driftb)) if driftb else mu

    c = mx + mu
    cb = mx + mub
    Ccum = np.cumsum(c, axis=1, dtype=np.float64)
    Ccumb = np.cumsum(cb, axis=1, dtype=np.float64)
    estop = np.exp(trans[STOP, :K].astype(np.float64))
    estop32 = estop.astype(np.float32)

    n_ = np.arange(1, D + 1)
    gfall = np.exp(feats - c[:, :, None])
    gball = np.exp(feats - cb[:, :, None])

    ga = np.zeros((B, D, KF), dtype=np.float32)
    livef = n_[None, :] <= L[:, None]
    ga[:, :, :K] = np.where(livef[:, :, None], gfall[:, :D, :], 0.0)
    holdon = n_[None, :] >= (L[:, None] + 1)
    ga[:, :, HOLD] = np.where(holdon, 1.0, 0.0)

    def x_rows(j):
        bjm1, bj = BND[j - 1], BND[j]
        act = L > bj
        tau = bjm1 + n_
        g = np.where(
            act[:, None, None], gfall[:, np.clip(tau - 1, 0, T - 1), :], 0.0
        ).astype(np.float32)
        return g, act

    def y_rows(j):
        bjm1, bj = BND[j - 1], BND[j]
        act = L >= bjm1 + 1
        sstar = np.maximum(bj - L, 0)
        g = np.zeros((B, D, KF), dtype=np.float32)
        tau = bj - n_
        valid = (
            (n_[None, :] >= np.maximum(sstar, 1)[:, None])
            & (n_[None, :] <= D - 1)
            & act[:, None]
        )
        gv = gball[:, np.clip(tau - 1, 0, T - 1), :]
        g[:, :, :K] = np.where(valid[:, :, None], gv, 0.0)
        g[act, D - 1, :K] = 1.0
        srcon = (n_[None, :] < sstar[:, None]) & act[:, None]
        g[:, :, SRCL] = np.where(srcon, 1.0, 0.0)
        u0 = np.zeros((B, KF), dtype=np.float32)
        inj = act & (sstar == 0)
        u0[inj, :K] = gball[inj, bj - 1, :] * estop32[None, :]
        u0[act & (sstar > 0), SRCL] = 1.0
        return g, u0

    def b_rows():
        bm1 = BND[M - 1]
        act = L >= bm1 + 1
        sstar = bm1 + 1 + D - L
        g = np.zeros((B, D, KF), dtype=np.float32)
        tau = bm1 + 1 + D - n_
        valid = (
            (n_[None, :] >= sstar[:, None]) & (n_[None, :] <= D - 1) & act[:, None]
        )
        gv = gball[:, np.clip(tau - 1, 0, T - 1), :]
        g[:, :, :K] = np.where(valid[:, :, None], gv, 0.0)
        g[act, D - 1, :K] = 1.0
        srcon = (n_[None, :] < sstar[:, None]) & act[:, None]
        g[:, :, SRCL] = np.where(srcon, 1.0, 0.0)
        u0 = np.zeros((B, KF), dtype=np.float32)
        u0[act, SRCL] = 1.0
        return g, u0

    xs = {}
    ys = {}
    acts = {}
    for j in range(2, M):
        xs[j], acts[j] = x_rows(j)
        ys[j] = y_rows(j)
    gb_, u0b = b_rows()

    def pack(gf, gbk):
        g = np.zeros((B, D, K2), dtype=np.float32)
        g[:, :, : gf.shape[2]] = gf
        g[:, :, KF : KF + gbk.shape[2]] = gbk
        return g

    # compacted column assignment: each column hosts one active fwd
    # half-chain (rows 0:49) and one active bwd half-chain (rows 49:98)
    e_start = np.zeros(KF, dtype=np.float32)
    e_start[START] = 1.0
    ones48 = np.zeros(KF, dtype=np.float32)
    ones48[:K] = 1.0
    WTOT = 2 * W
    per_core = []
    maps = []
    for cix in range(NCORES):
        bsl = range(cix * BC, (cix + 1) * BC)
        emis = np.zeros((K2, D, WTOT), dtype=np.float32)
        w0 = np.zeros((K2, WTOT), dtype=np.float32)
        fmap = []
        bmap = []
        for b in bsl:
            i = len(fmap)
            emis[:KF, :, i] = ga[b].T
            w0[:KF, i] = e_start
            fmap.append(("a", 0, b))
        for j in range(2, M):
            for b in bsl:
                if L[b] > BND[j]:
                    i = len(fmap)
                    emis[:K, :, i] = xs[j][b].T
                    w0[:KF, i] = ones48
                    fmap.append(("x", j, b))
        for j in range(2, M):
            for b in bsl:
                if L[b] >= BND[j - 1] + 1:
                    i = len(bmap)
                    emis[KF:, :, i] = ys[j][0][b].T
                    w0[KF:, i] = ys[j][1][b]
                    bmap.append(("y", j, b))
        for b in bsl:
            if L[b] > BND[M - 1]:
                i = len(bmap)
                emis[KF:, :, i] = gb_[b].T
                w0[KF:, i] = u0b[b]
                bmap.append(("b", 0, b))
        assert len(fmap) <= WTOT and len(bmap) <= WTOT, (len(fmap), len(bmap))
        emis8 = emis.astype(ml_dtypes.float8_e4m3fn)
        per_core.append(
            {
                "emisA": np.ascontiguousarray(emis8[:, :, 0:W]),
                "emisB": np.ascontiguousarray(emis8[:, :, W:WTOT]),
                "w0": np.ascontiguousarray(w0.astype(ml_dtypes.bfloat16)),
            }
        )
        maps.append((fmap, bmap))

    S_ = np.zeros((K2, K2), dtype=np.float32)
    S_[:K, :K] = np.exp(trans).T
    S_[:K, HOLD] = estop32
    S_[HOLD, HOLD] = 1.0
    S_[KF : KF + K, KF : KF + K] = np.exp(trans)
    S_[KF + SRCL, KF : KF + K] = estop32
    S_[KF + SRCL, KF + SRCL] = 1.0
    etil = S_.astype(ml_dtypes.bfloat16)

    ar = np.arange(B)
    ctx = {
        "seq_len": L,
        "estop": estop,
        "C_at_L": Ccum[ar, L - 1],
        "Cal": Ccum[:, BND[1] - 1],
        "Cx": {j: Ccum[:, BND[j] - 1] - Ccum[:, BND[j - 1] - 1] for j in range(2, M)},
        "CyL": {j: Ccumb[ar, L - 1] - Ccumb[:, BND[j - 1] - 1] for j in range(2, M)},
        "Cb": Ccumb[ar, L - 1] - Ccumb[:, BND[M - 1] - 1],
        "maps": maps,
    }
    return per_core, etil, ctx


def _combine(parts, ctx):
    """parts: (alpha [KF,B], xv {j:[K,B]}, yv {j:[K,B]}, bv [K,B]) f64."""
    L = ctx["seq_len"]
    estop = ctx["estop"]
    alpha, xv, yv, bv = parts

    d = {2: (yv[2] * alpha[:K, :]).sum(0)}
    n = {2: yv[2].sum(0)}
    for j in range(3, M):
        d[j] = (yv[j] * xv[j - 1]).sum(0)
        n[j] = yv[j].sum(0)
    d_b = (bv * xv[M - 1]).sum(0)

    scores = np.zeros(B)
    J1 = L <= BND[1] - 1
    scores[J1] = np.log(alpha[HOLD, J1]) + ctx["C_at_L"][J1]
    JM_ = L == BND[1]
    if JM_.any():
        dm = (alpha[:K, :] * estop[:, None]).sum(0)
        scores[JM_] = np.log(dm[JM_]) + ctx["Cal"][JM_]
    with np.errstate(divide="ignore", invalid="ignore"):
        for J in range(2, M):
            msk = (L > BND[J - 1]) & (L <= BND[J])
            sc = np.log(d[J]) + ctx["Cal"] + ctx["CyL"][J]
            for j in range(2, J):
                sc = sc + np.log(d[j]) - np.log(n[j]) + ctx["Cx"][j]
            scores[msk] = sc[msk]
        mskM = L > BND[M - 1]
        sc = np.log(d_b) + ctx["Cal"] + ctx["Cb"]
        for j in range(2, M):
            sc = sc + np.log(d[j]) - np.log(n[j]) + ctx["Cx"][j]
        scores[mskM] = sc[mskM]
    return scores


def _gold_score(feats, tags, seq_len, trans):
    feats = np.asarray(feats, dtype=np.float32)
    tags = np.asarray(tags, dtype=np.int64)
    seq_len = np.asarray(seq_len, dtype=np.int64)
    trans = np.asarray(trans, dtype=np.float32)
    tags_ext = np.concatenate(
        [np.full((B, 1), START, dtype=np.int64), tags], axis=1
    )
    trans_sc = trans[tags_ext[:, 1:], tags_ext[:, :-1]]
    emit_sc = np.take_along_axis(feats, tags_ext[:, 1:, None], axis=2)[..., 0]
    mask = np.arange(T)[None, :] < seq_len[:, None]
    last_tag = np.take_along_axis(tags_ext, seq_len[:, None], axis=1)[:, 0]
    gold = (
        np.where(mask, trans_sc + emit_sc, 0.0).sum(1, dtype=np.float64)
        + trans[STOP, last_tag]
    )
    return gold  # [B] f64


def kernel(feats, tags, seq_len, transitions):
    feats = np.asarray(feats)
    per_core, etil, ctx = _host_prep(feats, seq_len, transitions)
    nc = _build_module()
    in_maps = [{"etil": etil, **per_core[c]} for c in range(NCORES)]
    res = run_bass_kernel_spmd(nc, in_maps, list(range(NCORES)))
    outs = [np.asarray(res.results[c]["uout"]).astype(np.float64) for c in range(NCORES)]
    alpha = np.zeros((KF, B))
    xv = {j: np.zeros((K, B)) for j in range(2, M)}
    yv = {j: np.zeros((K, B)) for j in range(2, M)}
    bvv = np.zeros((K, B))
    for c in range(NCORES):
        fmap, bmap = ctx["maps"][c]
        o = outs[c]
        for i, (kind, j, b) in enumerate(fmap):
            if kind == "a":
                alpha[:, b] = o[:KF, i]
            else:
                xv[j][:, b] = o[:K, i]
        for i, (kind, j, b) in enumerate(bmap):
            if kind == "y":
                yv[j][:, b] = o[KF : KF + K, i]
            else:
                bvv[:, b] = o[KF : KF + K, i]
    scores = _combine((alpha, xv, yv, bvv), ctx)
    gold = _gold_score(feats, tags, seq_len, transitions)
    loss = np.mean(scores - gold)
    return np.float32(loss)
